# revision 16
# baseline (speedup 1.0000x reference)
"""Pointer-attention kernel for Trainium2 (8 NeuronCores, data-parallel over batch).

Computes, for P = pointer_input [B, S, R], weights W1/W2 [2R]:
    scores = P @ W1[:R] + (h @ W1[R:])[:, None]      # h-term is constant over S
    a      = softmax(scores, axis=S)                 #   -> cancels in softmax
    c      = einsum('bsr,bs->br', P, a)
    pi     = P @ W2[:R] + (c @ W2[R:])[:, None]

Math (exact):
    s1[b,s]  = P[b,s,:] . w1p          (w1p = W1[:R])
    E        = exp(s1)                 (softmax shift cancels; inputs are O(1))
    Z[b]     = sum_s E[b,s]
    craw[b,:]= sum_s E[b,s] * P[b,s,:]
    g[b]     = (craw[b,:] . w2c) / Z[b]            (w2c = W2[R:])
    pi[b,s]  = P[b,s,:] . w2p + g[b]               (w2p = W2[:R])

h_t and W1[R:] never affect the output. One pass over P.

Engine plan (v3, all-bf16 on chip):
  - P streams HBM->SBUF via SWDGE DMA with inline fp32->bf16 cast.
  - Both R-dots per s-tile start from one big 2x-mode bf16 DVE multiply
    per chunk.  The 512-element reduction is then routed per tile:
      ACT class:  ScalarE activation(Identity) + accum_out.
      GPS class:  GpSimd tensor_tensor fold 512->256, DVE folds ->64,
                  then one 3D DVE tensor_reduce for a run of tiles.
      DVE class:  all folds + reduce on DVE.
  - craw is computed column-wise on TensorE: lhsT = P-tile r-quarter
    [128s x 128r], rhs = exp(s1) column, accumulating [128r x 1] PSUM
    columns; the epilogue dot craw.w2c becomes a tiny DVE STT + one
    ones-matmul partition reduction batched over all 8 local batches.
"""

import numpy as np

B, S, R = 64, 2048, 512
N_CORES = 8
B_LOC = B // N_CORES          # 8 batches per core
P_PART = 128                  # partitions per s-tile
NT = S // P_PART              # 16 s-tiles per batch
CH = 8                        # s-tiles per DMA chunk
NCH = NT // CH                # 2 chunks per batch
RQ = R // P_PART              # 4 r-quarters (craw columns)

# --- routing knobs: per-batch tile index t in 0..NT-1, per dot kind ---
# t < _ACT_N         -> ScalarE accumulate route
# _ACT_N <= t < _GPS_END -> GpSimd fold1 + DVE fold2/3 + pool
# t >= _GPS_END      -> all-DVE folds + pool
S1_ACT_N = 8
S1_GPS_END = 16   # "GPS" run label kept; it is the DVE fold-tree route
PW2_ACT_N = 7
PW2_GPS_END = 16

_CACHED_NC = None


def _runs_for_chunk(act_n, gps_end, c):
    """(act_run, gps_run, dve_run) as (j0, n) within chunk c; j = t - c*CH."""
    t0, t1 = c * CH, (c + 1) * CH
    a0, a1 = t0, min(t1, act_n)
    g0, g1 = max(t0, act_n), min(t1, gps_end)
    d0, d1 = max(t0, gps_end), t1
    return (
        (a0 - t0, max(0, a1 - a0)),
        (g0 - t0, max(0, g1 - g0)),
        (d0 - t0, max(0, d1 - d0)),
    )


def _build_nc(b_loc=B_LOC, nt=NT, finalize=True):
    import concourse.bacc as bacc
    import concourse.bass as bass
    import concourse.mybir as mybir
    import concourse.tile as tile

    f32 = mybir.dt.float32
    bf16 = mybir.dt.bfloat16
    s_loc = nt * P_PART
    nch = nt // CH
    nc = bacc.Bacc(None, target_bir_lowering=False, debug=True)

    p_h = nc.declare_dram_parameter("p", [b_loc, s_loc, R], f32, isOutput=False)
    w1_h = nc.declare_dram_parameter("w1", [2 * R], f32, isOutput=False)
    w2_h = nc.declare_dram_parameter("w2", [2 * R], f32, isOutput=False)
    out_h = nc.declare_dram_parameter("out", [b_loc, s_loc], f32, isOutput=True)

    mult = mybir.AluOpType.mult

    def bcast_ap(src_ap, parts):
        # replicate a 1-D DRAM slice across `parts` partitions
        return bass.AP(
            tensor=src_ap.tensor,
            offset=src_ap.offset,
            ap=[[0, parts]] + [list(d) for d in src_ap.ap],
        )

    def rep_mid(src_ap, n):
        # [128, R] -> [128, n, R] via stride-0 middle dim
        return bass.AP(
            tensor=src_ap.tensor,
            offset=src_ap.offset,
            ap=[list(src_ap.ap[0]), [0, n], list(src_ap.ap[1])],
        )

    with tile.TileContext(nc) as tc:
        with (
            tc.tile_pool(name="consts", bufs=1) as consts,
            tc.tile_pool(name="prods", bufs=2) as prods,
            tc.tile_pool(name="folds", bufs=2) as folds,
            tc.tile_pool(name="scr", bufs=3) as scr,
            tc.tile_pool(name="perb", bufs=3) as perb,
            tc.tile_pool(name="epil", bufs=1) as epil,
            tc.tile_pool(name="smalls", bufs=2) as smalls,
            tc.tile_pool(name="psum_c", bufs=2, space="PSUM") as psum_c,
            tc.tile_pool(name="psum_s", bufs=2, space="PSUM") as psum_s,
        ):
            # ---- constants ----
            w1p_bf = consts.tile([P_PART, R], bf16)
            nc.gpsimd.dma_start(out=w1p_bf[:], in_=bcast_ap(w1_h[0:R], P_PART))
            w2p_bf = consts.tile([P_PART, R], bf16)
            nc.gpsimd.dma_start(out=w2p_bf[:], in_=bcast_ap(w2_h[0:R], P_PART))
            w2c_row = consts.tile([1, R], f32)
            nc.gpsimd.dma_start(out=w2c_row[:], in_=bcast_ap(w2_h[R : 2 * R], 1))
            ones_col = consts.tile([P_PART, 1], f32)
            nc.vector.memset(ones_col[:], 1.0)
            ones_row = consts.tile([1, P_PART], f32)
            nc.vector.memset(ones_row[:], 1.0)

            # ---- persistent per-core tiles ----
            es_all = epil.tile([P_PART, b_loc], f32)     # per-batch E row sums
            dq_row = epil.tile([1, b_loc], f32)          # craw.w2c dots
            pw2_all = epil.tile([P_PART, b_loc, nt], f32)
            pi_tiles = [
                epil.tile([P_PART, nt], f32, name=f"pi_{i}") for i in range(b_loc)
            ]
            g_all = epil.tile([P_PART, b_loc], f32)

            # all of P in SBUF as bf16: 128 tiles x 512, cast in-flight by
            # the SWDGE DMA.  Issued upfront with no ring-reuse waits so the
            # DMA engines stream at full HBM rate, decoupled from GpSimd's
            # fold work.
            pb_all = epil.tile([P_PART, b_loc * nt, R], bf16)
            for b in range(b_loc):
                src3 = p_h[b].rearrange("(t p) r -> p t r", p=P_PART)
                for c in range(nch):
                    nc.gpsimd.dma_start(
                        out=pb_all[:, b * nt + c * CH : b * nt + (c + 1) * CH, :],
                        in_=src3[:, c * CH : (c + 1) * CH, :],
                    )

            for b in range(b_loc):
                c_ps = psum_c.tile([1, R], f32, tag="c_ps")
                s1_b = perb.tile([P_PART, nt], f32, tag="s1_b")
                e_b = perb.tile([P_PART, nt], bf16, tag="e_b")
                pw2_b = pw2_all[:, b, :]

                for c in range(nch):
                    pb = pb_all[:, b * nt + c * CH : b * nt + (c + 1) * CH, :]

                    for kind, w_bf, out_cols, act_n, gps_end in (
                        ("s1", w1p_bf, s1_b, S1_ACT_N, S1_GPS_END),
                        ("pw2", w2p_bf, pw2_b, PW2_ACT_N, PW2_GPS_END),
                    ):
                        (a0, an), (g0, gn), (d0, dn) = _runs_for_chunk(
                            act_n, gps_end, c
                        )
                        prod = prods.tile(
                            [P_PART, CH, R], bf16, tag=f"prod_{kind}"
                        )
                        nc.vector.tensor_mul(
                            prod[:], pb, rep_mid(w_bf[:], CH)
                        )
                        # --- ACT route: per-tile Identity accumulate ---
                        for k in range(a0, a0 + an):
                            t = c * CH + k
                            scra = scr.tile([P_PART, R], bf16, tag="scra")
                            nc.scalar.activation(
                                out=scra[:],
                                in_=prod[:, k, :],
                                func=mybir.ActivationFunctionType.Identity,
                                bias=0.0,
                                scale=1.0,
                                accum_out=out_cols[:, t : t + 1],
                            )
                        # --- DVE fold-tree route: 2x bf16 folds 512->64,
                        # then one segmented 1x reduce for the whole run ---
                        fn = gn + dn
                        if fn:
                            f0 = g0
                            f1 = folds.tile(
                                [P_PART, CH, R // 2], bf16, tag=f"f1_{kind}"
                            )
                            nc.vector.tensor_add(
                                f1[:, f0 : f0 + fn, :],
                                prod[:, f0 : f0 + fn, 0 : R // 2],
                                prod[:, f0 : f0 + fn, R // 2 : R],
                            )
                            f2 = folds.tile(
                                [P_PART, CH, R // 4], bf16, tag=f"f2_{kind}"
                            )
                            nc.vector.tensor_add(
                                f2[:, f0 : f0 + fn, :],
                                f1[:, f0 : f0 + fn, 0 : R // 4],
                                f1[:, f0 : f0 + fn, R // 4 : R // 2],
                            )
                            # inner dim padded to 72 so the [fn, 64] AP cannot
                            # coalesce (reduce needs the window dim preserved)
                            f3 = folds.tile(
                                [P_PART, CH, R // 8 + 8], bf16, tag=f"f3_{kind}"
                            )
                            nc.vector.tensor_add(
                                f3[:, f0 : f0 + fn, 0 : R // 8],
                                f2[:, f0 : f0 + fn, 0 : R // 8],
                                f2[:, f0 : f0 + fn, R // 8 : R // 4],
                            )
                            t0 = c * CH + f0
                            nc.vector.reduce_sum(
                                out_cols[:, t0 : t0 + fn],
                                f3[:, f0 : f0 + fn, 0 : R // 8],
                                axis=mybir.AxisListType.X,
                            )
                    # --- exp for this chunk's 8 scores: e = exp(64 * s1/64)
                    nc.scalar.activation(
                        out=e_b[:, c * CH : (c + 1) * CH],
                        in_=s1_b[:, c * CH : (c + 1) * CH],
                        func=mybir.ActivationFunctionType.Exp,
                    )
                    # --- craw row on TensorE: lhsT = e column, rhs = P tile
                    for j in range(CH):
                        t = c * CH + j
                        nc.tensor.matmul(
                            c_ps[:],
                            lhsT=e_b[:, t : t + 1],
                            rhs=pb[:, j, :],
                            start=(t == 0),
                            stop=(t == nt - 1),
                        )

                # --- per-batch epilogue pieces (tiny) ---
                nc.vector.reduce_sum(
                    es_all[:, b : b + 1], e_b[:], axis=mybir.AxisListType.X
                )
                dqs = smalls.tile([1, R], f32, tag="dqs")
                nc.vector.scalar_tensor_tensor(
                    out=dqs[:],
                    in0=c_ps[:],
                    scalar=1.0,
                    in1=w2c_row[:],
                    op0=mult,
                    op1=mult,
                    accum_out=dq_row[:, b : b + 1],
                )

            # ---- batched epilogue over all 8 batches ----
            z_row = psum_s.tile([1, b_loc], f32, tag="z_row")
            nc.tensor.matmul(
                z_row[:], lhsT=ones_col[:], rhs=es_all[:], start=True, stop=True
            )
            zr = smalls.tile([1, b_loc], f32, tag="zr")
            nc.vector.reciprocal(out=zr[:], in_=z_row[:])
            g_row = smalls.tile([1, b_loc], f32, tag="g_row")
            nc.vector.tensor_mul(g_row[:], dq_row[:], zr[:])
            g_ps = psum_s.tile([P_PART, b_loc], f32, tag="g_ps")
            nc.tensor.matmul(
                g_ps[:], lhsT=ones_row[:], rhs=g_row[:], start=True, stop=True
            )
            nc.vector.tensor_copy(g_all[:], g_ps[:])

            for b in range(b_loc):
                pi_b = scr.tile([P_PART, nt], f32, tag="pi_b")
                # pw2 columns hold sum/64 -> pi = pw2*64 + g
                nc.vector.tensor_scalar_add(
                    pi_b[:], pw2_all[:, b, :], g_all[:, b : b + 1]
                )
                nc.sync.dma_start(
                    out=out_h[b].rearrange("(t p) -> p t", p=P_PART),
                    in_=pi_b[:],
                )

    if finalize:
        nc.finalize()
    return nc


def _get_nc():
    global _CACHED_NC
    if _CACHED_NC is None:
        _CACHED_NC = _build_nc()
    return _CACHED_NC


def run_sharded(pointer_input, W1, W2, trace=False, trace_kwargs=None):
    """Run the SPMD kernel; returns (full_output [1,B,S], BassKernelResults)."""
    from concourse.bass_utils import run_bass_kernel_spmd

    nc = _get_nc()
    pointer_input = np.ascontiguousarray(pointer_input, dtype=np.float32)
    W1 = np.ascontiguousarray(W1, dtype=np.float32)
    W2 = np.ascontiguousarray(W2, dtype=np.float32)
    in_maps = [
        {
            "p": pointer_input[i * B_LOC : (i + 1) * B_LOC],
            "w1": W1,
            "w2": W2,
        }
        for i in range(N_CORES)
    ]
    kw = dict(trace_kwargs or {})
    res = run_bass_kernel_spmd(
        nc, in_maps, list(range(N_CORES)), trace=trace, **kw
    )
    out = np.concatenate([res.results[i]["out"] for i in range(N_CORES)], axis=0)
    return out[None].astype(np.float32), res


def kernel(pointer_input, h_t, W1, W2):
    # h_t only shifts scores by a per-batch constant, which softmax cancels;
    # it does not affect the output.
    out, _ = run_sharded(pointer_input, W1, W2, trace=False)
    return out


# revision 18
# speedup vs baseline: 1.0140x; 1.0140x over previous
"""Pointer-attention kernel for Trainium2 (8 NeuronCores, data-parallel over batch).

Computes, for P = pointer_input [B, S, R], weights W1/W2 [2R]:
    scores = P @ W1[:R] + (h @ W1[R:])[:, None]      # h-term is constant over S
    a      = softmax(scores, axis=S)                 #   -> cancels in softmax
    c      = einsum('bsr,bs->br', P, a)
    pi     = P @ W2[:R] + (c @ W2[R:])[:, None]

Math (exact):
    s1[b,s]  = P[b,s,:] . w1p          (w1p = W1[:R])
    E        = exp(s1)                 (softmax shift cancels; inputs are O(1))
    Z[b]     = sum_s E[b,s]
    craw[b,:]= sum_s E[b,s] * P[b,s,:]
    g[b]     = (craw[b,:] . w2c) / Z[b]            (w2c = W2[R:])
    pi[b,s]  = P[b,s,:] . w2p + g[b]               (w2p = W2[:R])

h_t and W1[R:] never affect the output. One pass over P.

Engine plan (v3, all-bf16 on chip):
  - P streams HBM->SBUF via SWDGE DMA with inline fp32->bf16 cast.
  - Both R-dots per s-tile start from one big 2x-mode bf16 DVE multiply
    per chunk.  The 512-element reduction is then routed per tile:
      ACT class:  ScalarE activation(Identity) + accum_out.
      GPS class:  GpSimd tensor_tensor fold 512->256, DVE folds ->64,
                  then one 3D DVE tensor_reduce for a run of tiles.
      DVE class:  all folds + reduce on DVE.
  - craw is computed column-wise on TensorE: lhsT = P-tile r-quarter
    [128s x 128r], rhs = exp(s1) column, accumulating [128r x 1] PSUM
    columns; the epilogue dot craw.w2c becomes a tiny DVE STT + one
    ones-matmul partition reduction batched over all 8 local batches.
"""

import numpy as np

B, S, R = 64, 2048, 512
N_CORES = 8
B_LOC = B // N_CORES          # 8 batches per core
P_PART = 128                  # partitions per s-tile
NT = S // P_PART              # 16 s-tiles per batch
CH = 8                        # s-tiles per DMA chunk
NCH = NT // CH                # 2 chunks per batch
RQ = R // P_PART              # 4 r-quarters (craw columns)

# --- routing knobs: per-batch tile index t in 0..NT-1, per dot kind ---
# t < _ACT_N         -> ScalarE accumulate route
# _ACT_N <= t < _GPS_END -> GpSimd fold1 + DVE fold2/3 + pool
# t >= _GPS_END      -> all-DVE folds + pool
S1_ACT_N = 8
S1_GPS_END = 16   # "GPS" run label kept; it is the DVE fold-tree route
PW2_ACT_N = 7
PW2_GPS_END = 16

_CACHED_NC = None


def _runs_for_chunk(act_n, gps_end, c):
    """(act_run, gps_run, dve_run) as (j0, n) within chunk c; j = t - c*CH."""
    t0, t1 = c * CH, (c + 1) * CH
    a0, a1 = t0, min(t1, act_n)
    g0, g1 = max(t0, act_n), min(t1, gps_end)
    d0, d1 = max(t0, gps_end), t1
    return (
        (a0 - t0, max(0, a1 - a0)),
        (g0 - t0, max(0, g1 - g0)),
        (d0 - t0, max(0, d1 - d0)),
    )


def _build_nc(b_loc=B_LOC, nt=NT, finalize=True):
    import concourse.bacc as bacc
    import concourse.bass as bass
    import concourse.mybir as mybir
    import concourse.tile as tile

    f32 = mybir.dt.float32
    bf16 = mybir.dt.bfloat16
    s_loc = nt * P_PART
    nch = nt // CH
    nc = bacc.Bacc(None, target_bir_lowering=False, debug=True)

    p_h = nc.declare_dram_parameter("p", [b_loc, s_loc, R], f32, isOutput=False)
    w1_h = nc.declare_dram_parameter("w1", [2 * R], f32, isOutput=False)
    w2_h = nc.declare_dram_parameter("w2", [2 * R], f32, isOutput=False)
    out_h = nc.declare_dram_parameter("out", [b_loc, s_loc], f32, isOutput=True)

    mult = mybir.AluOpType.mult

    def bcast_ap(src_ap, parts):
        # replicate a 1-D DRAM slice across `parts` partitions
        return bass.AP(
            tensor=src_ap.tensor,
            offset=src_ap.offset,
            ap=[[0, parts]] + [list(d) for d in src_ap.ap],
        )

    def rep_mid(src_ap, n):
        # [128, R] -> [128, n, R] via stride-0 middle dim
        return bass.AP(
            tensor=src_ap.tensor,
            offset=src_ap.offset,
            ap=[list(src_ap.ap[0]), [0, n], list(src_ap.ap[1])],
        )

    with tile.TileContext(nc) as tc:
        with (
            tc.tile_pool(name="consts", bufs=1) as consts,
            tc.tile_pool(name="prods", bufs=2) as prods,
            tc.tile_pool(name="folds", bufs=2) as folds,
            tc.tile_pool(name="scr", bufs=3) as scr,
            tc.tile_pool(name="perb", bufs=3) as perb,
            tc.tile_pool(name="epil", bufs=1) as epil,
            tc.tile_pool(name="smalls", bufs=2) as smalls,
            tc.tile_pool(name="psum_c", bufs=2, space="PSUM") as psum_c,
            tc.tile_pool(name="psum_s", bufs=2, space="PSUM") as psum_s,
        ):
            # ---- constants ----
            w1p_bf = consts.tile([P_PART, R], bf16)
            nc.gpsimd.dma_start(out=w1p_bf[:], in_=bcast_ap(w1_h[0:R], P_PART))
            w2p_bf = consts.tile([P_PART, R], bf16)
            nc.gpsimd.dma_start(out=w2p_bf[:], in_=bcast_ap(w2_h[0:R], P_PART))
            w2c_row = consts.tile([1, R], f32)
            nc.gpsimd.dma_start(out=w2c_row[:], in_=bcast_ap(w2_h[R : 2 * R], 1))
            ones_col = consts.tile([P_PART, 1], f32)
            nc.vector.memset(ones_col[:], 1.0)
            ones_row = consts.tile([1, P_PART], f32)
            nc.vector.memset(ones_row[:], 1.0)

            # ---- persistent per-core tiles ----
            es_all = epil.tile([P_PART, b_loc], f32)     # per-batch E row sums
            dq_row = epil.tile([1, b_loc], f32)          # craw.w2c dots
            pw2_all = epil.tile([P_PART, b_loc, nt], f32)
            pi_all = epil.tile([P_PART, b_loc * nt], f32)
            g_all = epil.tile([P_PART, b_loc], f32)

            # all of P in SBUF as bf16: 128 tiles x 512, cast in-flight by
            # the SWDGE DMA.  Issued upfront with no ring-reuse waits so the
            # DMA engines stream at full HBM rate, decoupled from GpSimd's
            # fold work.
            pb_all = epil.tile([P_PART, b_loc * nt, R], bf16)
            for b in range(b_loc):
                src3 = p_h[b].rearrange("(t p) r -> p t r", p=P_PART)
                for c in range(nch):
                    nc.gpsimd.dma_start(
                        out=pb_all[:, b * nt + c * CH : b * nt + (c + 1) * CH, :],
                        in_=src3[:, c * CH : (c + 1) * CH, :],
                    )

            for b in range(b_loc):
                c_ps = psum_c.tile([1, R], f32, tag="c_ps")
                s1_b = perb.tile([P_PART, nt], f32, tag="s1_b")
                e_b = perb.tile([P_PART, nt], bf16, tag="e_b")
                pw2_b = pw2_all[:, b, :]

                for c in range(nch):
                    pb = pb_all[:, b * nt + c * CH : b * nt + (c + 1) * CH, :]

                    for kind, w_bf, out_cols, act_n, gps_end in (
                        ("s1", w1p_bf, s1_b, S1_ACT_N, S1_GPS_END),
                        ("pw2", w2p_bf, pw2_b, PW2_ACT_N, PW2_GPS_END),
                    ):
                        (a0, an), (g0, gn), (d0, dn) = _runs_for_chunk(
                            act_n, gps_end, c
                        )
                        prod = prods.tile(
                            [P_PART, CH, R], bf16, tag=f"prod_{kind}"
                        )
                        nc.vector.tensor_mul(
                            prod[:], pb, rep_mid(w_bf[:], CH)
                        )
                        # --- ACT route: per-tile Identity accumulate ---
                        for k in range(a0, a0 + an):
                            t = c * CH + k
                            scra = scr.tile([P_PART, R], bf16, tag="scra")
                            nc.scalar.activation(
                                out=scra[:],
                                in_=prod[:, k, :],
                                func=mybir.ActivationFunctionType.Identity,
                                bias=0.0,
                                scale=1.0,
                                accum_out=out_cols[:, t : t + 1],
                            )
                        # --- DVE fold-tree route: 2x bf16 folds 512->64,
                        # then one segmented 1x reduce for the whole run ---
                        fn = gn + dn
                        if fn:
                            f0 = g0
                            f1 = folds.tile(
                                [P_PART, CH, R // 2], bf16, tag=f"f1_{kind}"
                            )
                            nc.vector.tensor_add(
                                f1[:, f0 : f0 + fn, :],
                                prod[:, f0 : f0 + fn, 0 : R // 2],
                                prod[:, f0 : f0 + fn, R // 2 : R],
                            )
                            f2 = folds.tile(
                                [P_PART, CH, R // 4], bf16, tag=f"f2_{kind}"
                            )
                            nc.vector.tensor_add(
                                f2[:, f0 : f0 + fn, :],
                                f1[:, f0 : f0 + fn, 0 : R // 4],
                                f1[:, f0 : f0 + fn, R // 4 : R // 2],
                            )
                            # inner dim padded to 72 so the [fn, 64] AP cannot
                            # coalesce (reduce needs the window dim preserved)
                            f3 = folds.tile(
                                [P_PART, CH, R // 8 + 8], bf16, tag=f"f3_{kind}"
                            )
                            nc.vector.tensor_add(
                                f3[:, f0 : f0 + fn, 0 : R // 8],
                                f2[:, f0 : f0 + fn, 0 : R // 8],
                                f2[:, f0 : f0 + fn, R // 8 : R // 4],
                            )
                            t0 = c * CH + f0
                            nc.vector.reduce_sum(
                                out_cols[:, t0 : t0 + fn],
                                f3[:, f0 : f0 + fn, 0 : R // 8],
                                axis=mybir.AxisListType.X,
                            )
                    # --- exp for this chunk's 8 scores: e = exp(64 * s1/64)
                    nc.scalar.activation(
                        out=e_b[:, c * CH : (c + 1) * CH],
                        in_=s1_b[:, c * CH : (c + 1) * CH],
                        func=mybir.ActivationFunctionType.Exp,
                    )
                    # --- craw row on TensorE: lhsT = e column, rhs = P tile
                    for j in range(CH):
                        t = c * CH + j
                        nc.tensor.matmul(
                            c_ps[:],
                            lhsT=e_b[:, t : t + 1],
                            rhs=pb[:, j, :],
                            start=(t == 0),
                            stop=(t == nt - 1),
                        )

                # --- per-batch epilogue pieces (tiny) ---
                nc.vector.reduce_sum(
                    es_all[:, b : b + 1], e_b[:], axis=mybir.AxisListType.X
                )
                dqs = smalls.tile([1, R], f32, tag="dqs")
                nc.vector.scalar_tensor_tensor(
                    out=dqs[:],
                    in0=c_ps[:],
                    scalar=1.0,
                    in1=w2c_row[:],
                    op0=mult,
                    op1=mult,
                    accum_out=dq_row[:, b : b + 1],
                )

            # ---- batched epilogue over all 8 batches ----
            z_row = psum_s.tile([1, b_loc], f32, tag="z_row")
            nc.tensor.matmul(
                z_row[:], lhsT=ones_col[:], rhs=es_all[:], start=True, stop=True
            )
            zr = smalls.tile([1, b_loc], f32, tag="zr")
            nc.vector.reciprocal(out=zr[:], in_=z_row[:])
            g_row = smalls.tile([1, b_loc], f32, tag="g_row")
            nc.vector.tensor_mul(g_row[:], dq_row[:], zr[:])
            g_ps = psum_s.tile([P_PART, b_loc], f32, tag="g_ps")
            nc.tensor.matmul(
                g_ps[:], lhsT=ones_row[:], rhs=g_row[:], start=True, stop=True
            )
            nc.vector.tensor_copy(g_all[:], g_ps[:])

            for b in range(b_loc):
                pi_b = scr.tile([P_PART, nt], f32, tag="pi_b")
                # pw2 columns hold sum/64 -> pi = pw2*64 + g
                nc.vector.tensor_scalar_add(
                    pi_b[:], pw2_all[:, b, :], g_all[:, b : b + 1]
                )
                nc.sync.dma_start(
                    out=out_h[b].rearrange("(t p) -> p t", p=P_PART),
                    in_=pi_b[:],
                )

    if finalize:
        nc.finalize()
    return nc


def _get_nc():
    global _CACHED_NC
    if _CACHED_NC is None:
        _CACHED_NC = _build_nc()
    return _CACHED_NC


def run_sharded(pointer_input, W1, W2, trace=False, trace_kwargs=None):
    """Run the SPMD kernel; returns (full_output [1,B,S], BassKernelResults)."""
    from concourse.bass_utils import run_bass_kernel_spmd

    nc = _get_nc()
    pointer_input = np.ascontiguousarray(pointer_input, dtype=np.float32)
    W1 = np.ascontiguousarray(W1, dtype=np.float32)
    W2 = np.ascontiguousarray(W2, dtype=np.float32)
    in_maps = [
        {
            "p": pointer_input[i * B_LOC : (i + 1) * B_LOC],
            "w1": W1,
            "w2": W2,
        }
        for i in range(N_CORES)
    ]
    kw = dict(trace_kwargs or {})
    res = run_bass_kernel_spmd(
        nc, in_maps, list(range(N_CORES)), trace=trace, **kw
    )
    out = np.concatenate([res.results[i]["out"] for i in range(N_CORES)], axis=0)
    return out[None].astype(np.float32), res


def kernel(pointer_input, h_t, W1, W2):
    # h_t only shifts scores by a per-batch constant, which softmax cancels;
    # it does not affect the output.
    out, _ = run_sharded(pointer_input, W1, W2, trace=False)
    return out


# revision 20
# speedup vs baseline: 1.3884x; 1.3692x over previous
"""Pointer-attention kernel for Trainium2 (8 NeuronCores, data-parallel over batch).

Computes, for P = pointer_input [B, S, R], weights W1/W2 [2R]:
    scores = P @ W1[:R] + (h @ W1[R:])[:, None]      # h-term is constant over S
    a      = softmax(scores, axis=S)                 #   -> cancels in softmax
    c      = einsum('bsr,bs->br', P, a)
    pi     = P @ W2[:R] + (c @ W2[R:])[:, None]

Math (exact):
    s1[b,s]  = P[b,s,:] . w1p          (w1p = W1[:R])
    E        = exp(s1)                 (softmax shift cancels; inputs are O(1))
    Z[b]     = sum_s E[b,s]
    craw[b,:]= sum_s E[b,s] * P[b,s,:]
    g[b]     = (craw[b,:] . w2c) / Z[b]            (w2c = W2[R:])
    pi[b,s]  = P[b,s,:] . w2p + g[b]               (w2p = W2[:R])

h_t and W1[R:] never affect the output. One pass over P.

Engine plan (v3, all-bf16 on chip):
  - P streams HBM->SBUF via SWDGE DMA with inline fp32->bf16 cast.
  - Both R-dots per s-tile start from one big 2x-mode bf16 DVE multiply
    per chunk.  The 512-element reduction is then routed per tile:
      ACT class:  ScalarE activation(Identity) + accum_out.
      GPS class:  GpSimd tensor_tensor fold 512->256, DVE folds ->64,
                  then one 3D DVE tensor_reduce for a run of tiles.
      DVE class:  all folds + reduce on DVE.
  - craw is computed column-wise on TensorE: lhsT = P-tile r-quarter
    [128s x 128r], rhs = exp(s1) column, accumulating [128r x 1] PSUM
    columns; the epilogue dot craw.w2c becomes a tiny DVE STT + one
    ones-matmul partition reduction batched over all 8 local batches.
"""

import numpy as np

B, S, R = 64, 2048, 512
N_CORES = 8
B_LOC = B // N_CORES          # 8 batches per core
P_PART = 128                  # partitions per s-tile
NT = S // P_PART              # 16 s-tiles per batch
CH = 8                        # s-tiles per DMA chunk
NCH = NT // CH                # 2 chunks per batch
RQ = R // P_PART              # 4 r-quarters (craw columns)

# --- routing knobs: per-batch tile index t in 0..NT-1, per dot kind ---
# t < _ACT_N         -> ScalarE accumulate route
# _ACT_N <= t < _GPS_END -> GpSimd fold1 + DVE fold2/3 + pool
# t >= _GPS_END      -> all-DVE folds + pool
S1_ACT_N = 8
S1_GPS_END = 16   # "GPS" run label kept; it is the DVE fold-tree route
PW2_ACT_N = 7
PW2_GPS_END = 16

_CACHED_NC = None


def _runs_for_chunk(act_n, gps_end, c):
    """(act_run, gps_run, dve_run) as (j0, n) within chunk c; j = t - c*CH."""
    t0, t1 = c * CH, (c + 1) * CH
    a0, a1 = t0, min(t1, act_n)
    g0, g1 = max(t0, act_n), min(t1, gps_end)
    d0, d1 = max(t0, gps_end), t1
    return (
        (a0 - t0, max(0, a1 - a0)),
        (g0 - t0, max(0, g1 - g0)),
        (d0 - t0, max(0, d1 - d0)),
    )


def _build_nc(b_loc=B_LOC, nt=NT, finalize=True):
    import concourse.bacc as bacc
    import concourse.bass as bass
    import concourse.mybir as mybir
    import concourse.tile as tile

    f32 = mybir.dt.float32
    bf16 = mybir.dt.bfloat16
    s_loc = nt * P_PART
    nch = nt // CH
    nc = bacc.Bacc(None, target_bir_lowering=False, debug=True)

    p_h = nc.declare_dram_parameter("p", [b_loc, s_loc, R], f32, isOutput=False)
    w1_h = nc.declare_dram_parameter("w1", [2 * R], f32, isOutput=False)
    w2_h = nc.declare_dram_parameter("w2", [2 * R], f32, isOutput=False)
    out_h = nc.declare_dram_parameter("out", [b_loc, s_loc], f32, isOutput=True)

    mult = mybir.AluOpType.mult

    def bcast_ap(src_ap, parts):
        # replicate a 1-D DRAM slice across `parts` partitions
        return bass.AP(
            tensor=src_ap.tensor,
            offset=src_ap.offset,
            ap=[[0, parts]] + [list(d) for d in src_ap.ap],
        )

    def rep_mid(src_ap, n):
        # [128, R] -> [128, n, R] via stride-0 middle dim
        return bass.AP(
            tensor=src_ap.tensor,
            offset=src_ap.offset,
            ap=[list(src_ap.ap[0]), [0, n], list(src_ap.ap[1])],
        )

    with tile.TileContext(nc) as tc:
        with (
            tc.tile_pool(name="consts", bufs=1) as consts,
            tc.tile_pool(name="prods", bufs=2) as prods,
            tc.tile_pool(name="folds", bufs=2) as folds,
            tc.tile_pool(name="scr", bufs=3) as scr,
            tc.tile_pool(name="perb", bufs=3) as perb,
            tc.tile_pool(name="epil", bufs=1) as epil,
            tc.tile_pool(name="smalls", bufs=2) as smalls,
            tc.tile_pool(name="psum_c", bufs=2, space="PSUM") as psum_c,
            tc.tile_pool(name="psum_s", bufs=2, space="PSUM") as psum_s,
        ):
            # ---- constants ----
            w1p_bf = consts.tile([P_PART, R], bf16)
            nc.gpsimd.dma_start(out=w1p_bf[:], in_=bcast_ap(w1_h[0:R], P_PART))
            w2p_bf = consts.tile([P_PART, R], bf16)
            nc.gpsimd.dma_start(out=w2p_bf[:], in_=bcast_ap(w2_h[0:R], P_PART))
            w2c_row = consts.tile([1, R], f32)
            nc.gpsimd.dma_start(out=w2c_row[:], in_=bcast_ap(w2_h[R : 2 * R], 1))
            ones_col = consts.tile([P_PART, 1], f32)
            nc.vector.memset(ones_col[:], 1.0)
            ones_row = consts.tile([1, P_PART], f32)
            nc.vector.memset(ones_row[:], 1.0)

            # ---- persistent per-core tiles ----
            es_all = epil.tile([P_PART, b_loc], f32)     # per-batch E row sums
            dq_row = epil.tile([1, b_loc], f32)          # craw.w2c dots
            pw2_all = epil.tile([P_PART, b_loc, nt], f32)
            pi_all = epil.tile([P_PART, b_loc * nt], f32)
            g_all = epil.tile([P_PART, b_loc], f32)

            # all of P in SBUF as bf16: 128 tiles x 512, cast in-flight by
            # the SWDGE DMA.  Issued upfront with no ring-reuse waits so the
            # DMA engines stream at full HBM rate, decoupled from GpSimd's
            # fold work.
            pb_all = epil.tile([P_PART, b_loc * nt, R], bf16)
            for b in range(b_loc):
                src3 = p_h[b].rearrange("(t p) r -> p t r", p=P_PART)
                for c in range(nch):
                    nc.gpsimd.dma_start(
                        out=pb_all[:, b * nt + c * CH : b * nt + (c + 1) * CH, :],
                        in_=src3[:, c * CH : (c + 1) * CH, :],
                    )

            for b in range(b_loc):
                c_ps = psum_c.tile([1, R], f32, tag="c_ps")
                s1_b = perb.tile([P_PART, nt], f32, tag="s1_b")
                e_b = perb.tile([P_PART, nt], bf16, tag="e_b")
                pw2_b = pw2_all[:, b, :]

                for c in range(nch):
                    pb = pb_all[:, b * nt + c * CH : b * nt + (c + 1) * CH, :]

                    for kind, w_bf, out_cols, act_n, gps_end in (
                        ("s1", w1p_bf, s1_b, S1_ACT_N, S1_GPS_END),
                        ("pw2", w2p_bf, pw2_b, PW2_ACT_N, PW2_GPS_END),
                    ):
                        (a0, an), (g0, gn), (d0, dn) = _runs_for_chunk(
                            act_n, gps_end, c
                        )
                        prod = prods.tile(
                            [P_PART, CH, R], bf16, tag=f"prod_{kind}"
                        )
                        nc.vector.tensor_mul(
                            prod[:], pb, rep_mid(w_bf[:], CH)
                        )
                        # --- ACT route: per-tile Identity accumulate ---
                        for k in range(a0, a0 + an):
                            t = c * CH + k
                            scra = scr.tile([P_PART, R], bf16, tag="scra")
                            nc.scalar.activation(
                                out=scra[:],
                                in_=prod[:, k, :],
                                func=mybir.ActivationFunctionType.Identity,
                                bias=0.0,
                                scale=1.0,
                                accum_out=out_cols[:, t : t + 1],
                            )
                        # --- DVE fold-tree route: 2x bf16 folds 512->64,
                        # then one segmented 1x reduce for the whole run ---
                        fn = gn + dn
                        if fn:
                            f0 = g0
                            f1 = folds.tile(
                                [P_PART, CH, R // 2], bf16, tag=f"f1_{kind}"
                            )
                            nc.vector.tensor_add(
                                f1[:, f0 : f0 + fn, :],
                                prod[:, f0 : f0 + fn, 0 : R // 2],
                                prod[:, f0 : f0 + fn, R // 2 : R],
                            )
                            f2 = folds.tile(
                                [P_PART, CH, R // 4], bf16, tag=f"f2_{kind}"
                            )
                            nc.vector.tensor_add(
                                f2[:, f0 : f0 + fn, :],
                                f1[:, f0 : f0 + fn, 0 : R // 4],
                                f1[:, f0 : f0 + fn, R // 4 : R // 2],
                            )
                            # inner dim padded to 72 so the [fn, 64] AP cannot
                            # coalesce (reduce needs the window dim preserved)
                            f3 = folds.tile(
                                [P_PART, CH, R // 8 + 8], bf16, tag=f"f3_{kind}"
                            )
                            nc.vector.tensor_add(
                                f3[:, f0 : f0 + fn, 0 : R // 8],
                                f2[:, f0 : f0 + fn, 0 : R // 8],
                                f2[:, f0 : f0 + fn, R // 8 : R // 4],
                            )
                            t0 = c * CH + f0
                            nc.vector.reduce_sum(
                                out_cols[:, t0 : t0 + fn],
                                f3[:, f0 : f0 + fn, 0 : R // 8],
                                axis=mybir.AxisListType.X,
                            )
                    # --- exp for this chunk's 8 scores: e = exp(64 * s1/64)
                    nc.scalar.activation(
                        out=e_b[:, c * CH : (c + 1) * CH],
                        in_=s1_b[:, c * CH : (c + 1) * CH],
                        func=mybir.ActivationFunctionType.Exp,
                    )
                    # --- craw row on TensorE: lhsT = e column, rhs = P tile
                    for j in range(CH):
                        t = c * CH + j
                        nc.tensor.matmul(
                            c_ps[:],
                            lhsT=e_b[:, t : t + 1],
                            rhs=pb[:, j, :],
                            start=(t == 0),
                            stop=(t == nt - 1),
                        )

                # --- per-batch epilogue pieces (tiny) ---
                nc.vector.reduce_sum(
                    es_all[:, b : b + 1], e_b[:], axis=mybir.AxisListType.X
                )
                dqs = smalls.tile([1, R], f32, tag="dqs")
                nc.vector.scalar_tensor_tensor(
                    out=dqs[:],
                    in0=c_ps[:],
                    scalar=1.0,
                    in1=w2c_row[:],
                    op0=mult,
                    op1=mult,
                    accum_out=dq_row[:, b : b + 1],
                )

            # ---- batched epilogue over all 8 batches ----
            z_row = psum_s.tile([1, b_loc], f32, tag="z_row")
            nc.tensor.matmul(
                z_row[:], lhsT=ones_col[:], rhs=es_all[:], start=True, stop=True
            )
            zr = smalls.tile([1, b_loc], f32, tag="zr")
            nc.vector.reciprocal(out=zr[:], in_=z_row[:])
            g_row = smalls.tile([1, b_loc], f32, tag="g_row")
            nc.vector.tensor_mul(g_row[:], dq_row[:], zr[:])
            g_ps = psum_s.tile([P_PART, b_loc], f32, tag="g_ps")
            nc.tensor.matmul(
                g_ps[:], lhsT=ones_row[:], rhs=g_row[:], start=True, stop=True
            )
            nc.vector.tensor_copy(g_all[:], g_ps[:])

            for b in range(b_loc):
                nc.vector.tensor_scalar_add(
                    pi_all[:, b * nt : (b + 1) * nt],
                    pw2_all[:, b, :],
                    g_all[:, b : b + 1],
                )
            # one 64 KiB store; out_h viewed as [p, b, t] with per-partition
            # contiguous 512 B runs; run_sharded un-permutes on the host
            out_flat = bass.AP(
                tensor=out_h[0, 0:1].tensor,
                offset=0,
                ap=[[b_loc * nt, P_PART], [1, b_loc * nt]],
            )
            nc.sync.dma_start(out=out_flat, in_=pi_all[:])

    if finalize:
        nc.finalize()
    return nc


def _get_nc():
    global _CACHED_NC
    if _CACHED_NC is None:
        _CACHED_NC = _build_nc()
    return _CACHED_NC


def run_sharded(pointer_input, W1, W2, trace=False, trace_kwargs=None):
    """Run the SPMD kernel; returns (full_output [1,B,S], BassKernelResults)."""
    from concourse.bass_utils import run_bass_kernel_spmd

    nc = _get_nc()
    pointer_input = np.ascontiguousarray(pointer_input, dtype=np.float32)
    W1 = np.ascontiguousarray(W1, dtype=np.float32)
    W2 = np.ascontiguousarray(W2, dtype=np.float32)
    in_maps = [
        {
            "p": pointer_input[i * B_LOC : (i + 1) * B_LOC],
            "w1": W1,
            "w2": W2,
        }
        for i in range(N_CORES)
    ]
    kw = dict(trace_kwargs or {})
    res = run_bass_kernel_spmd(
        nc, in_maps, list(range(N_CORES)), trace=trace, **kw
    )
    outs = []
    for i in range(N_CORES):
        raw = np.asarray(res.results[i]["out"]).reshape(P_PART, B_LOC, NT)
        outs.append(raw.transpose(1, 2, 0).reshape(B_LOC, S))
    out = np.concatenate(outs, axis=0)
    return out[None].astype(np.float32), res


def kernel(pointer_input, h_t, W1, W2):
    # h_t only shifts scores by a per-batch constant, which softmax cancels;
    # it does not affect the output.
    out, _ = run_sharded(pointer_input, W1, W2, trace=False)
    return out
